# revision 1
# baseline (speedup 1.0000x reference)
"""DGCNN (2x EdgeConv kNN=5 + MLP head) Trainium2 kernel, data-parallel over 8 NeuronCores.

Contract: kernel(**inputs) takes the FULL inputs of nn_DEC_41180146434796
(pos [32,2048,3] + MLP weights) and returns the FULL [32,2] output.
Each core processes 4 graphs end-to-end (kNN, gathers, max-aggregations local).
"""
import numpy as np

import concourse.bass as bass
import concourse.mybir as mybir
from concourse import bacc, tile
from concourse import bass_utils
from concourse.masks import make_identity

F32 = mybir.dt.float32
F32R = mybir.dt.float32r
U32 = mybir.dt.uint32
I16 = mybir.dt.int16
AF = mybir.ActivationFunctionType
ALU = mybir.AluOpType
AX = mybir.AxisListType

N = 2048          # nodes per graph
NG = 4            # graphs per core
K = 5             # kNN neighbors (incl self)
NT = 16           # node tiles of 128
NCORES = 8

_CACHE = {}


def _sigma_read(ap):
    """View a [C, 2048] natural-ordered tensor so its free stream is sigma-ordered.

    sigma col s = 16*q + b  <->  node i = 128*b + q.  Iteration (q outer, b inner),
    address = 128*b + q.
    """
    return ap.rearrange("c (b q) -> c q b", b=16, q=128)


def build_nc():
    nc = bacc.Bacc(None, target_bir_lowering=False)

    # ---------------- I/O ----------------
    posT_d = nc.dram_tensor("posT", [NG, 3, N], F32, kind="ExternalInput")
    # folded weights (see kernel() for host-side folding)
    w1a_A_d = nc.dram_tensor("w1aA", [3, 64], F32, kind="ExternalInput")
    w1a_B_d = nc.dram_tensor("w1aB", [3, 64], F32, kind="ExternalInput")
    w1b_d = nc.dram_tensor("w1b", [64, 64], F32, kind="ExternalInput")
    w1c_d = nc.dram_tensor("w1c", [64, 64], F32, kind="ExternalInput")
    w2A_d = nc.dram_tensor("w2A", [64, 128], F32, kind="ExternalInput")
    w2B_d = nc.dram_tensor("w2B", [64, 128], F32, kind="ExternalInput")
    wl1_d = nc.dram_tensor("wl1", [64, 1024], F32, kind="ExternalInput")
    wl2_d = nc.dram_tensor("wl2", [128, 1024], F32, kind="ExternalInput")
    wm1_d = nc.dram_tensor("wm1", [128, 8, 512], F32, kind="ExternalInput")
    wm2_d = nc.dram_tensor("wm2", [128, 4, 256], F32, kind="ExternalInput")
    wout_d = nc.dram_tensor("wout", [128, 2, 2], F32, kind="ExternalInput")
    # biases / scales, per-partition layouts
    b1a_d = nc.dram_tensor("b1a", [64, 1], F32, kind="ExternalInput")
    b1b_d = nc.dram_tensor("b1b", [64, 1], F32, kind="ExternalInput")
    b1c_d = nc.dram_tensor("b1c", [64, 1], F32, kind="ExternalInput")
    s1c_d = nc.dram_tensor("s1c", [64, 1], F32, kind="ExternalInput")
    h1c_d = nc.dram_tensor("h1c", [64, 1], F32, kind="ExternalInput")
    b2_d = nc.dram_tensor("b2", [128, 1], F32, kind="ExternalInput")
    bl_d = nc.dram_tensor("bl", [128, 8], F32, kind="ExternalInput")
    bm1_d = nc.dram_tensor("bm1", [128, 4], F32, kind="ExternalInput")
    bm2_d = nc.dram_tensor("bm2", [128, 2], F32, kind="ExternalInput")
    bout_d = nc.dram_tensor("bout", [2, 1], F32, kind="ExternalInput")

    out_d = nc.dram_tensor("out", [2, NG], F32, kind="ExternalOutput")

    with tile.TileContext(nc) as tc:
        with tc.tile_pool(name="wpool", bufs=1) as wp, \
             tc.tile_pool(name="persist", bufs=1) as pp, \
             tc.tile_pool(name="work", bufs=1) as work, \
             tc.tile_pool(name="workB", bufs=2) as workB, \
             tc.tile_pool(name="ps", bufs=2, space="PSUM") as psp:

            # ---------------- weights to SBUF (one-time) ----------------
            def wload(dram, shape, dtype=F32R, name=None):
                t = wp.tile(shape, dtype, name=name or dram.name + "_s")
                if dtype == F32R:
                    t0 = work.tile(shape, F32, tag="wstg", name=(name or dram.name) + "_stg")
                    nc.sync.dma_start(t0[:], dram[:])
                    nc.vector.tensor_copy(t[:], t0[:])
                else:
                    nc.sync.dma_start(t[:], dram[:])
                return t

            w1aA = wload(w1a_A_d, [3, 64])
            w1aB = wload(w1a_B_d, [3, 64])
            w1b = wload(w1b_d, [64, 64])
            w1c = wload(w1c_d, [64, 64])
            w2A = wload(w2A_d, [64, 128])
            w2B = wload(w2B_d, [64, 128])
            wl1 = wload(wl1_d, [64, 1024])
            wl2 = wload(wl2_d, [128, 1024])
            wm1 = wload(wm1_d, [128, 8, 512])
            wm2 = wload(wm2_d, [128, 4, 256])
            wout = wload(wout_d, [128, 2, 2])
            b1a = wload(b1a_d, [64, 1], F32)
            b1b = wload(b1b_d, [64, 1], F32)
            b1c = wload(b1c_d, [64, 1], F32)
            s1c = wload(s1c_d, [64, 1], F32)
            h1c = wload(h1c_d, [64, 1], F32)
            b2 = wload(b2_d, [128, 1], F32)
            bl = wload(bl_d, [128, 8], F32)
            bm1 = wload(bm1_d, [128, 4], F32)
            bm2 = wload(bm2_d, [128, 2], F32)
            bout = wload(bout_d, [2, 1], F32)

            ident = wp.tile([128, 128], F32)
            make_identity(nc, ident[:])
            ones3 = wp.tile([3, 1], F32)
            nc.vector.memset(ones3[:], 1.0)
            ones64 = wp.tile([64, 1], F32)
            nc.vector.memset(ones64[:], 1.0)
            onesrow = wp.tile([1, N], F32R)
            nc.vector.memset(onesrow[:].bitcast(F32), 1.0)
            negones = wp.tile([1, N], F32R)
            nc.vector.memset(negones[:].bitcast(F32), -1.0)

            # pooled & relu'd features for the head: [128, mt(8), graph(4)]
            poolr = pp.tile([128, 8, NG], F32R)

            # ============ per-graph pipeline ============
            for g in range(NG):
                # ---- S0: load pos, round to f32r ----
                posT0 = work.tile([3, N], F32, tag="scrA")
                nc.sync.dma_start(posT0[:], posT_d[g])
                posTr = work.tile([3, N], F32R, tag="posTr")
                nc.vector.tensor_copy(posTr[:], posT0[:])

                # ---- S1: norms ----
                sq = work.tile([3, N], F32, tag="scrA")
                nc.scalar.activation(sq[:], posTr[:].bitcast(F32), AF.Square)
                x2p = psp.tile([1, N], F32, tag="ps")
                for c in range(4):
                    nc.tensor.matmul(x2p[:, 512 * c:512 * (c + 1)], ones3[:],
                                     sq[:, 512 * c:512 * (c + 1)])
                x2s = work.tile([1, N], F32R, tag="x2s")
                nc.vector.tensor_copy(x2s[:], x2p[:])
                negx2 = work.tile([1, N], F32R, tag="negx2")
                nc.scalar.activation(negx2[:], x2s[:].bitcast(F32), AF.Copy, scale=-1.0)

                # ---- S2: augmented gram operands [5, N] ----
                rhsA = work.tile([5, N], F32R, tag="rhsA")
                nc.scalar.activation(rhsA[0:3, :], posTr[:].bitcast(F32), AF.Copy)
                nc.sync.dma_start(rhsA[3:4, :], x2s[:])
                nc.sync.dma_start(rhsA[4:5, :], onesrow[:])
                lhsA = work.tile([5, N], F32R, tag="lhsA")
                nc.scalar.activation(lhsA[0:3, :], posTr[:].bitcast(F32), AF.Copy, scale=2.0)
                nc.sync.dma_start(lhsA[3:4, :], negones[:])
                nc.sync.dma_start(lhsA[4:5, :], negx2[:])

                # ---- S3: gram1 + topk1 ----
                idxall1 = work.tile([128, NT, 8], U32, tag="idxall")
                for t in range(NT):
                    ps = psp.tile([128, N], F32, tag="ps")
                    for c in range(4):
                        nc.tensor.matmul(ps[:, 512 * c:512 * (c + 1)],
                                         lhsA[:, 128 * t:128 * (t + 1)],
                                         rhsA[:, 512 * c:512 * (c + 1)])
                    vals = work.tile([128, 8], F32, tag="vals")
                    nc.vector.max(out=vals[:], in_=ps[:])
                    nc.vector.max_index(out=idxall1[:, t, :], in_max=vals[:], in_values=ps[:])

                # ---- S4: redistribute indices -> wrapped i16 [64, 640] ----
                wrap1 = _make_wrap(nc, tc, work, psp, ident, idxall1, ngroups=4, tag="w1")

                # ---- S5: B1 (natural) and A1 (sigma) node features ----
                B1T = work.tile([64, N], F32, tag="BT")
                psb = psp.tile([64, N], F32, tag="ps")
                for c in range(4):
                    nc.tensor.matmul(psb[:, 512 * c:512 * (c + 1)], w1aB[:],
                                     posTr[:, 512 * c:512 * (c + 1)])
                nc.scalar.activation(B1T[:], psb[:], AF.Copy)
                A1s = work.tile([64, N], F32, tag="As")
                psa = psp.tile([64, N], F32, tag="ps")
                sig_pos = _sigma_read(posTr[:])
                for c in range(4):
                    nc.tensor.matmul(psa[:, 512 * c:512 * (c + 1)], w1aA[:],
                                     sig_pos[:, 32 * c:32 * (c + 1), :])
                nc.scalar.activation(A1s[:], psa[:], AF.Copy)

                # ---- S6+S7: conv1 MLP over 5 neighbor slabs ----
                macc = work.tile([64, N], F32, tag="macc")
                for k in range(K):
                    g1 = workB.tile([64, N], F32, tag="gslab")
                    nc.gpsimd.ap_gather(
                        out_ap=g1[:].unsqueeze(-1), in_ap=B1T[:].unsqueeze(-1),
                        idxs_ap=wrap1[:, 128 * k:128 * (k + 1)],
                        channels=64, num_elems=N, d=1, num_idxs=N)
                    nc.vector.tensor_tensor(out=g1[:], in0=g1[:], in1=A1s[:], op=ALU.add)
                    r1a = work.tile([64, N], F32R, tag="r1aslab")
                    nc.scalar.activation(r1a[:], g1[:], AF.Relu, bias=b1a[:])
                    ps1b = psp.tile([64, N], F32, tag="ps")
                    for c in range(4):
                        nc.tensor.matmul(ps1b[:, 512 * c:512 * (c + 1)], w1b[:],
                                         r1a[:, 512 * c:512 * (c + 1)])
                    r1b = work.tile([64, N], F32R, tag="r1bslab")
                    nc.scalar.activation(r1b[:], ps1b[:], AF.Relu, bias=b1b[:])
                    ps1c = psp.tile([64, N], F32, tag="ps")
                    for c in range(4):
                        nc.tensor.matmul(ps1c[:, 512 * c:512 * (c + 1)], w1c[:],
                                         r1b[:, 512 * c:512 * (c + 1)])
                    if k == 0:
                        nc.vector.tensor_copy(macc[:], ps1c[:])
                    else:
                        nc.vector.tensor_tensor(out=macc[:], in0=macc[:], in1=ps1c[:], op=ALU.max)

                # ---- x1 = bn(relu(macc + b1c)) written natural-order ----
                t1 = work.tile([64, N], F32, tag="scrA")
                nc.scalar.activation(t1[:], macc[:], AF.Relu, bias=b1c[:])
                x1nat = work.tile([64, N], F32R, tag="x1nat")
                nc.vector.tensor_scalar(
                    out=_sigma_read(x1nat[:]),
                    in0=t1[:].rearrange("c (q b) -> c q b", q=128, b=16),
                    scalar1=s1c[:], scalar2=h1c[:],
                    op0=ALU.mult, op1=ALU.add)

                # ---- S8: conv2 norms (natural) ----
                sq1 = work.tile([64, N], F32, tag="scrA")
                nc.scalar.activation(sq1[:], x1nat[:].bitcast(F32), AF.Square)
                x2p2 = psp.tile([1, N], F32, tag="ps")
                for c in range(4):
                    nc.tensor.matmul(x2p2[:, 512 * c:512 * (c + 1)], ones64[:],
                                     sq1[:, 512 * c:512 * (c + 1)])
                x2c = work.tile([1, N], F32R, tag="x2s")
                nc.vector.tensor_copy(x2c[:], x2p2[:])
                negx2c = work.tile([1, N], F32R, tag="negx2")
                nc.scalar.activation(negx2c[:], x2c[:].bitcast(F32), AF.Copy, scale=-1.0)

                # ---- S9: aug operands [66, N] ----
                rhsA2 = work.tile([66, N], F32R, tag="rhsA")
                nc.scalar.activation(rhsA2[0:64, :], x1nat[:].bitcast(F32), AF.Copy)
                nc.sync.dma_start(rhsA2[64:65, :], x2c[:])
                nc.sync.dma_start(rhsA2[65:66, :], onesrow[:])
                lhsA2 = work.tile([66, N], F32R, tag="lhsA")
                nc.scalar.activation(lhsA2[0:64, :], x1nat[:].bitcast(F32), AF.Copy, scale=2.0)
                nc.sync.dma_start(lhsA2[64:65, :], negones[:])
                nc.sync.dma_start(lhsA2[65:66, :], negx2c[:])

                # ---- S10: gram2 + topk2 ----
                idxall2 = work.tile([128, NT, 8], U32, tag="idxall")
                for t in range(NT):
                    ps = psp.tile([128, N], F32, tag="ps")
                    for c in range(4):
                        nc.tensor.matmul(ps[:, 512 * c:512 * (c + 1)],
                                         lhsA2[:, 128 * t:128 * (t + 1)],
                                         rhsA2[:, 512 * c:512 * (c + 1)])
                    vals2 = work.tile([128, 8], F32, tag="vals")
                    nc.vector.max(out=vals2[:], in_=ps[:])
                    nc.vector.max_index(out=idxall2[:, t, :], in_max=vals2[:], in_values=ps[:])

                # ---- S11: redistribute ----
                wrap2 = _make_wrap(nc, tc, work, psp, ident, idxall2, ngroups=8, tag="w2")

                # ---- S12: B2 (natural), A2 (sigma) ----
                B2T = work.tile([128, N], F32, tag="BT")
                psb2 = psp.tile([128, N], F32, tag="ps")
                for c in range(4):
                    nc.tensor.matmul(psb2[:, 512 * c:512 * (c + 1)], w2B[:],
                                     x1nat[:, 512 * c:512 * (c + 1)])
                nc.scalar.activation(B2T[:], psb2[:], AF.Copy)
                A2s = work.tile([128, N], F32, tag="As")
                psa2 = psp.tile([128, N], F32, tag="ps")
                sig_x1 = _sigma_read(x1nat[:])
                for c in range(4):
                    nc.tensor.matmul(psa2[:, 512 * c:512 * (c + 1)], w2A[:],
                                     sig_x1[:, 32 * c:32 * (c + 1), :])
                nc.scalar.activation(A2s[:], psa2[:], AF.Copy)

                # ---- S13+S14: gather-max + combine ----
                macc2 = work.tile([128, N], F32, tag="macc")
                for k in range(K):
                    g2 = workB.tile([128, N], F32, tag="gslab")
                    nc.gpsimd.ap_gather(
                        out_ap=g2[:].unsqueeze(-1), in_ap=B2T[:].unsqueeze(-1),
                        idxs_ap=wrap2[:, 128 * k:128 * (k + 1)],
                        channels=128, num_elems=N, d=1, num_idxs=N)
                    if k == 0:
                        nc.vector.tensor_copy(macc2[:], g2[:])
                    else:
                        nc.vector.tensor_tensor(out=macc2[:], in0=macc2[:], in1=g2[:], op=ALU.max)
                nc.vector.tensor_tensor(out=macc2[:], in0=macc2[:], in1=A2s[:], op=ALU.add)
                x2sg = work.tile([128, N], F32R, tag="x2sg")
                nc.scalar.activation(x2sg[:], macc2[:], AF.Relu, bias=b2[:])

                # ---- S15: linear-l + global max pool ----
                for mt in range(8):
                    psl = psp.tile([128, N], F32, tag="ps")
                    for c in range(4):
                        nc.tensor.matmul(psl[:, 512 * c:512 * (c + 1)],
                                         wl1[:, 128 * mt:128 * (mt + 1)],
                                         sig_x1[:, 32 * c:32 * (c + 1), :],
                                         start=True, stop=False)
                    for c in range(4):
                        nc.tensor.matmul(psl[:, 512 * c:512 * (c + 1)],
                                         wl2[:, 128 * mt:128 * (mt + 1)],
                                         x2sg[:, 512 * c:512 * (c + 1)],
                                         start=False, stop=True)
                    pr = work.tile([128, 1], F32, tag="poolred")
                    nc.vector.tensor_reduce(pr[:], psl[:], axis=AX.X, op=ALU.max)
                    nc.scalar.activation(poolr[:, mt, g:g + 1], pr[:],
                                         AF.Relu, bias=bl[:, mt:mt + 1])

            # ============ head MLP (all graphs) ============
            rm1 = pp.tile([128, 4, NG], F32R)
            for mt in range(4):
                ph = psp.tile([128, NG], F32, tag="ps")
                for kk in range(8):
                    nc.tensor.matmul(ph[:], wm1[:, kk, 128 * mt:128 * (mt + 1)],
                                     poolr[:, kk, :], start=(kk == 0), stop=(kk == 7))
                nc.scalar.activation(rm1[:, mt, :], ph[:], AF.Relu,
                                     bias=bm1[:, mt:mt + 1])
            rm2 = pp.tile([128, 2, NG], F32R)
            for mt in range(2):
                ph = psp.tile([128, NG], F32, tag="ps")
                for kk in range(4):
                    nc.tensor.matmul(ph[:], wm2[:, kk, 128 * mt:128 * (mt + 1)],
                                     rm1[:, kk, :], start=(kk == 0), stop=(kk == 3))
                nc.scalar.activation(rm2[:, mt, :], ph[:], AF.Relu,
                                     bias=bm2[:, mt:mt + 1])
            pho = psp.tile([2, NG], F32, tag="ps")
            for kk in range(2):
                nc.tensor.matmul(pho[:], wout[:, kk, :], rm2[:, kk, :],
                                 start=(kk == 0), stop=(kk == 1))
            outs = pp.tile([2, NG], F32)
            nc.vector.tensor_scalar_add(outs[:], pho[:], bout[:])
            nc.sync.dma_start(out_d[:], outs[:])

    nc.compile()
    return nc


def _make_wrap(nc, tc, work, psp, ident, idxall, ngroups, tag):
    """[128, 16, 8] u32 find_index8 results -> wrapped i16 [16*ngroups, 640] for ap_gather.

    Edge order m = 2048*k + 16*q + b: node i = 128*b + q, slot k.
    """
    F32_ = mybir.dt.float32
    I16_ = mybir.dt.int16
    idxf = work.tile([128, 5, 16], F32_, tag=tag + "idxf")
    nc.vector.tensor_copy(idxf[:], idxall[:, :, 0:5].transpose([0, 2, 1]))
    tp = psp.tile([80, 128], F32_, tag="ps")
    nc.tensor.transpose(tp[:], idxf[:].rearrange("p a b -> p (a b)"), ident[:])
    idxt16 = work.tile([80, 128], I16_, tag=tag + "idxt16")
    nc.vector.tensor_copy(idxt16[:], tp[:])
    wrap = work.tile([16 * ngroups, 640], I16_, tag=tag + "wrap")
    for gg in range(ngroups):
        for k in range(5):
            nc.sync.dma_start(wrap[16 * gg:16 * (gg + 1), 128 * k:128 * (k + 1)],
                              idxt16[16 * k:16 * k + 16, :])
    return wrap


def _fold_weights(inp):
    """Host-side BN folding / edge-weight splitting. Layout-only + tiny weight algebra."""
    f = {k: np.asarray(v, dtype=np.float64) for k, v in inp.items()}
    w = {}
    # conv1 layer a: e @ W1a = x_i @ (Wtop - Wbot) + x_j @ Wbot
    w["w1aA"] = (f["w1a"][:3] - f["w1a"][3:])
    w["w1aB"] = f["w1a"][3:]
    w["b1a"] = f["b1a"]
    # fold (s1a, h1a) into layer b; (s1b, h1b) into layer c
    w["w1b"] = f["s1a"][:, None] * f["w1b"]
    w["b1b"] = f["h1a"] @ f["w1b"] + f["b1b"]
    w["w1c"] = f["s1b"][:, None] * f["w1c"]
    w["b1c"] = f["h1b"] @ f["w1c"] + f["b1c"]
    w["s1c"], w["h1c"] = f["s1c"], f["h1c"]
    # conv2
    w["w2A"] = f["w2"][:64] - f["w2"][64:]
    w["w2B"] = f["w2"][64:]
    w["b2"] = f["b2"]
    # linear l: x1-part plain; x2-part folded with (s2, h2)
    wl1 = f["wl"][:64]
    wl2 = f["s2"][:, None] * f["wl"][64:]
    blf = f["bl"] + f["h2"] @ f["wl"][64:]
    w["wl1"], w["wl2"], w["bl"] = wl1, wl2, blf
    # head: fold (sl, hl) into m1; (sm1, hm1) into m2; (sm2, hm2) into out
    w["wm1"] = f["sl"][:, None] * f["wm1"]
    w["bm1"] = f["hl"] @ f["wm1"] + f["bm1"]
    w["wm2"] = f["sm1"][:, None] * f["wm2"]
    w["bm2"] = f["hm1"] @ f["wm2"] + f["bm2"]
    w["wout"] = f["sm2"][:, None] * f["wout"]
    w["bout"] = f["hm2"] @ f["wout"] + f["bout"]
    return {k: v.astype(np.float32) for k, v in w.items()}


def _weight_maps(w):
    m = {}
    m["w1aA"] = w["w1aA"]
    m["w1aB"] = w["w1aB"]
    m["w1b"] = w["w1b"]
    m["w1c"] = w["w1c"]
    m["w2A"] = w["w2A"]
    m["w2B"] = w["w2B"]
    m["wl1"] = w["wl1"]
    m["wl2"] = w["wl2"]
    m["wm1"] = np.ascontiguousarray(w["wm1"].reshape(8, 128, 512).transpose(1, 0, 2))
    m["wm2"] = np.ascontiguousarray(w["wm2"].reshape(4, 128, 256).transpose(1, 0, 2))
    m["wout"] = np.ascontiguousarray(w["wout"].reshape(2, 128, 2).transpose(1, 0, 2))
    m["b1a"] = w["b1a"].reshape(64, 1)
    m["b1b"] = w["b1b"].reshape(64, 1)
    m["b1c"] = w["b1c"].reshape(64, 1)
    m["s1c"] = w["s1c"].reshape(64, 1)
    m["h1c"] = w["h1c"].reshape(64, 1)
    m["b2"] = w["b2"].reshape(128, 1)
    m["bl"] = np.ascontiguousarray(w["bl"].reshape(8, 128).T)
    m["bm1"] = np.ascontiguousarray(w["bm1"].reshape(4, 128).T)
    m["bm2"] = np.ascontiguousarray(w["bm2"].reshape(2, 128).T)
    m["bout"] = w["bout"].reshape(2, 1)
    return {k: np.ascontiguousarray(v, dtype=np.float32) for k, v in m.items()}


def kernel(**inputs):
    if "nc" not in _CACHE:
        _CACHE["nc"] = build_nc()
    nc = _CACHE["nc"]

    w = _fold_weights(inputs)
    wm = _weight_maps(w)
    pos = np.asarray(inputs["pos"], dtype=np.float32)  # [32, 2048, 3]
    B = pos.shape[0]

    in_maps = []
    for c in range(NCORES):
        m = dict(wm)
        m["posT"] = np.ascontiguousarray(pos[NG * c:NG * (c + 1)].transpose(0, 2, 1))
        in_maps.append(m)

    res = bass_utils.run_bass_kernel_spmd(nc, in_maps, core_ids=list(range(NCORES)))
    out = np.zeros((B, 2), dtype=np.float32)
    for c in range(NCORES):
        out[NG * c:NG * (c + 1)] = res.results[c]["out"].T
    return out



# revision 2
# speedup vs baseline: 1.0570x; 1.0570x over previous
"""DGCNN (2x EdgeConv kNN=5 + MLP head) Trainium2 kernel, data-parallel over 8 NeuronCores.

v4: DVE does only the irreducible kNN top-k (Max/MaxIndex over SBUF scan tiles);
conv work is rebalanced onto Pool/ACT/PE and software-pipelined against the
topk blocks (T1(0) | T1(1)+C1(0) | T2(0)+C1(1) | T1(2)+C2(0) | ...). conv1
edge-MLP runs as 3 uniform 128-partition slab-pairs via per-core-group
indirect_copy gathers (self slab = const iota blocks).
"""
import numpy as np
from collections import deque

import concourse.bass as bass
import concourse.mybir as mybir
from concourse import bacc, tile
from concourse import bass_utils

F32 = mybir.dt.float32
F32R = mybir.dt.float32r
U32 = mybir.dt.uint32
U16 = mybir.dt.uint16
I16 = mybir.dt.int16
AF = mybir.ActivationFunctionType
ALU = mybir.AluOpType
AX = mybir.AxisListType

N = 2048          # nodes per graph
NG = 4            # graphs per core
K = 5             # kNN neighbors (incl self)
NT = 16           # node tiles of 128
NCORES = 8

_CACHE = {}


def _sigma_read(ap):
    """View a [C, 2048] natural-ordered tensor so its free stream is sigma-ordered.

    sigma col s = 16*q + b  <->  node i = 128*b + q.
    """
    return ap.rearrange("c (b q) -> c q b", b=16, q=128)


def build_nc():
    nc = bacc.Bacc(None, target_bir_lowering=False)

    # ---------------- I/O ----------------
    posT_d = nc.dram_tensor("posT", [NG, 3, N], F32, kind="ExternalInput")
    w1aA2_d = nc.dram_tensor("w1aA2", [3, 128], F32, kind="ExternalInput")   # [A|A]
    w1aB2_d = nc.dram_tensor("w1aB2", [3, 128], F32, kind="ExternalInput")   # [B|B]
    w1bB_d = nc.dram_tensor("w1bB", [128, 128], F32, kind="ExternalInput")   # blkdiag
    w1cB_d = nc.dram_tensor("w1cB", [128, 128], F32, kind="ExternalInput")   # blkdiag
    w2A_d = nc.dram_tensor("w2A", [64, 128], F32, kind="ExternalInput")
    w2B_d = nc.dram_tensor("w2B", [64, 128], F32, kind="ExternalInput")
    wl1_d = nc.dram_tensor("wl1", [64, 1024], F32, kind="ExternalInput")
    wl2_d = nc.dram_tensor("wl2", [128, 1024], F32, kind="ExternalInput")
    wm1_d = nc.dram_tensor("wm1", [128, 8, 512], F32, kind="ExternalInput")
    wm2_d = nc.dram_tensor("wm2", [128, 4, 256], F32, kind="ExternalInput")
    wout_d = nc.dram_tensor("wout", [128, 2, 2], F32, kind="ExternalInput")
    ident_d = nc.dram_tensor("ident", [128, 128], F32, kind="ExternalInput")
    pm3_d = nc.dram_tensor("pm3", [3, 2], F32, kind="ExternalInput")         # [1,-1] cols
    pm64_d = nc.dram_tensor("pm64", [64, 2], F32, kind="ExternalInput")
    iotaw_d = nc.dram_tensor("iotaw", [16, 128], I16, kind="ExternalInput")  # blk[b,q]=128b+q
    cst_d = nc.dram_tensor("cst", [2, N], F32R, kind="ExternalInput")         # [1; -1] rows
    # biases / scales, per-partition layouts
    b1a2_d = nc.dram_tensor("b1a2", [128, 1], F32, kind="ExternalInput")
    b1b2_d = nc.dram_tensor("b1b2", [128, 1], F32, kind="ExternalInput")
    b1c2_d = nc.dram_tensor("b1c2", [128, 1], F32, kind="ExternalInput")
    s1c_d = nc.dram_tensor("s1c", [64, 1], F32, kind="ExternalInput")
    h1c_d = nc.dram_tensor("h1c", [64, 1], F32, kind="ExternalInput")
    b2_d = nc.dram_tensor("b2", [128, 1], F32, kind="ExternalInput")
    bl_d = nc.dram_tensor("bl", [128, 8], F32, kind="ExternalInput")
    bm1_d = nc.dram_tensor("bm1", [128, 4], F32, kind="ExternalInput")
    bm2_d = nc.dram_tensor("bm2", [128, 2], F32, kind="ExternalInput")
    bout_d = nc.dram_tensor("bout", [2, 1], F32, kind="ExternalInput")

    out_d = nc.dram_tensor("out", [2, NG], F32, kind="ExternalOutput")

    with tile.TileContext(nc) as tc:
        with tc.tile_pool(name="wpool", bufs=1) as wp, \
             tc.tile_pool(name="persist", bufs=1) as pp, \
             tc.tile_pool(name="ps", bufs=2, space="PSUM") as psp:

            # ---------------- conv-path weights ----------------
            # f32 loads up-front (cheap DMAs); f32r conversions are deferred
            # into the first topk block via the chunk queue.
            wconv = []

            def wload(dram, shape, dtype=F32R, name=None):
                t = wp.tile(shape, dtype, name=name or dram.name + "_s")
                if dtype == F32R:
                    def conv(t=t, dram=dram, shape=shape, name=name):
                        t0 = wst.tile(shape, F32, tag="wstg", bufs=1,
                                      name=(name or dram.name) + "_stg")
                        nc.sync.dma_start(t0[:], dram[:])
                        nc.scalar.activation(t[:], t0[:], AF.Copy)
                    wconv.append(conv)
                else:
                    def ldf(t=t, dram=dram):
                        nc.sync.dma_start(t[:], dram[:])
                    wconv.append(ldf)
                return t

            # pm3 conversion is needed by prep(0) immediately: do it inline
            pm3 = wp.tile([3, 2], F32R, name="pm3_s")
            pm3_0 = wp.tile([3, 2], F32, name="pm3_stg")
            nc.sync.dma_start(pm3_0[:], pm3_d[:])
            nc.vector.tensor_copy(pm3[:], pm3_0[:])

            w1aA2 = wload(w1aA2_d, [3, 128])
            w1aB2 = wload(w1aB2_d, [3, 128])
            w1bB = wload(w1bB_d, [128, 128])
            w1cB = wload(w1cB_d, [128, 128])
            w2A = wload(w2A_d, [64, 128])
            w2B = wload(w2B_d, [64, 128])

            def wload_split(dram, shape, nsplit, name):
                t = wp.tile(shape, F32R, name=name)
                step = shape[1] // nsplit
                for j in range(nsplit):
                    def conv(t=t, dram=dram, j=j, step=step, shape=shape, name=name):
                        t0 = wst.tile([shape[0], step], F32, tag="wstg", bufs=1,
                                      name=f"{name}_stg{j}")
                        nc.sync.dma_start(t0[:], dram[:, step * j:step * (j + 1)])
                        nc.scalar.activation(t[:, step * j:step * (j + 1)], t0[:],
                                             AF.Copy)
                    wconv.append(conv)
                return t

            wl1 = wload_split(wl1_d, [64, 1024], 4, "wl1s")
            wl2 = wload_split(wl2_d, [128, 1024], 4, "wl2s")
            identr = wload(ident_d, [128, 128], name="identr")
            pm64 = wload(pm64_d, [64, 2], name="pm64_s")
            identf = wp.tile([128, 128], F32, name="identf")
            nc.sync.dma_start(identf[:], ident_d[:])
            b1a2 = wload(b1a2_d, [128, 1], F32)
            b1b2 = wload(b1b2_d, [128, 1], F32)
            b1c2 = wload(b1c2_d, [128, 1], F32)
            s1c = wload(s1c_d, [64, 1], F32)
            h1c = wload(h1c_d, [64, 1], F32)
            b2 = wload(b2_d, [128, 1], F32)
            bl = wload(bl_d, [128, 8], F32)

            # persistent ping-pong gram operands; first/last are const rows
            # lhsX = [-x2/2; x; -1], rhsX = [1; x; x2/2]
            lhs5 = [pp.tile([5, N], F32R, name=f"lhs5_{i}") for i in range(2)]
            rhs5 = [pp.tile([5, N], F32R, name=f"rhs5_{i}") for i in range(2)]
            lhs66 = [pp.tile([66, N], F32R, name=f"lhs66_{i}") for i in range(2)]
            rhs66 = [pp.tile([66, N], F32R, name=f"rhs66_{i}") for i in range(2)]
            def consts0():
                nc.sync.dma_start(lhs5[0][4:5, :], cst_d[1:2, :])
                nc.sync.dma_start(rhs5[0][0:1, :], cst_d[0:1, :])

            def lateconsts():
                nc.sync.dma_start(lhs5[1][4:5, :], cst_d[1:2, :])
                nc.sync.dma_start(rhs5[1][0:1, :], cst_d[0:1, :])
                for i in range(2):
                    nc.sync.dma_start(lhs66[i][65:66, :], cst_d[1:2, :])
                    nc.sync.dma_start(rhs66[i][0:1, :], cst_d[0:1, :])
            wconv.append(lateconsts)

            # conv1 pair-gather index tiles: [iota|k1], [k2|k3], [k4|iota]
            wpr = [pp.tile([128, 128], I16, name=f"wpr{p}") for p in range(3)]
            # conv2 slab gather indices: [iota, k1, k2, k3, k4]
            wrap2 = pp.tile([128, 5, 128], I16, name="wrap2")
            def iotainit():
                for j in range(4):
                    nc.sync.dma_start(wpr[0][16 * j:16 * (j + 1), :], iotaw_d[:])
                    nc.sync.dma_start(wpr[2][64 + 16 * j:80 + 16 * j, :], iotaw_d[:])
                for j in range(8):
                    nc.sync.dma_start(wrap2[16 * j:16 * (j + 1), 0, :], iotaw_d[:])
            wconv.insert(0, iotainit)

            # pooled & relu'd features for the head: [128, mt(8), graph(4)]
            poolr = pp.tile([128, 8, NG], F32R)

            with tc.tile_pool(name="work", bufs=1) as work, \
                 tc.tile_pool(name="workB", bufs=1) as workB, \
                 tc.tile_pool(name="wstage", bufs=1) as wst:
                # ============ software-pipelined per-graph streams ============
                st = [dict() for _ in range(NG)]

                def prep_chunks(g):
                    def c0():
                        posT0 = work.tile([3, N], F32, tag="posT0",
                                          name=f"posT0_{g}")
                        nc.sync.dma_start(posT0[:], posT_d[g])
                        posTr = work.tile([3, N], F32R, tag="posTr", bufs=2,
                                          name=f"posTr_{g}")
                        nc.scalar.activation(posTr[:], posT0[:], AF.Copy)
                        st[g]["posTr"] = posTr

                    def c1():
                        posTr = st[g]["posTr"]
                        L, R = lhs5[g % 2], rhs5[g % 2]
                        sq = work.tile([3, N], F32R, tag="posT0", name=f"sq_{g}")
                        nc.scalar.activation(sq[:], posTr[:].bitcast(F32), AF.Square)
                        x2pn = psp.tile([1, N], F32, tag="ps", name=f"x2pn_{g}")
                        for c in range(4):
                            nc.tensor.matmul(x2pn[:, 512 * c:512 * (c + 1)], pm3[:, 0:1],
                                             sq[:, 512 * c:512 * (c + 1)])
                        x2pp = psp.tile([1, N], F32, tag="ps", name=f"x2pp_{g}")
                        for c in range(4):
                            nc.tensor.matmul(x2pp[:, 512 * c:512 * (c + 1)], pm3[:, 1:2],
                                             sq[:, 512 * c:512 * (c + 1)])
                        nc.scalar.activation(L[0:1, :], x2pn[:], AF.Copy, scale=0.5)
                        x2r = work.tile([3, N], F32R, tag="posT0", name=f"x2r_{g}")
                        nc.scalar.activation(x2r[0:1, :], x2pp[:], AF.Copy, scale=0.5)
                        nc.sync.dma_start(R[4:5, :], x2r[0:1, :])
                        nc.sync.dma_start(L[1:4, :], posTr[:])
                        nc.sync.dma_start(R[1:4, :], posTr[:])
                        st[g]["lhs5"], st[g]["rhs5"] = L, R

                    return [c0, c1]

                def topk_emit(g, conv, lname, rname, iname):
                    """16 gram tiles -> ACT copy -> Max/MaxIndex; drains conv q."""
                    lhs, rhs = st[g][lname], st[g][rname]
                    idxall = work.tile([128, NT, 8], U32, tag="idxall", bufs=2,
                                       name=f"{iname}_{g}")
                    st[g][iname] = idxall
                    for t in range(NT):
                        ps = psp.tile([128, N], F32, tag="ps", name=f"{iname}ps{t}_{g}")
                        for c in range(4):
                            nc.tensor.matmul(ps[:, 512 * c:512 * (c + 1)],
                                             lhs[:, 128 * t:128 * (t + 1)],
                                             rhs[:, 512 * c:512 * (c + 1)])
                        scan = workB.tile([128, N], F32, tag="scan", bufs=3,
                                          name=f"{iname}scan{t}_{g}")
                        nc.scalar.activation(scan[:], ps[:], AF.Copy)
                        vals = work.tile([128, 8], F32, tag="vals", bufs=2,
                                         name=f"{iname}vals{t}_{g}")
                        nc.vector.max(out=vals[:], in_=scan[:])
                        nc.vector.max_index(out=idxall[:, t, :], in_max=vals[:],
                                            in_values=scan[:])
                        if conv:
                            conv.popleft()()

                def conv1_chunks(g):
                    chunks = []

                    def s45():
                        dests = [(wpr[0][64:128, :], 64, 128),
                                 (wpr[1][0:64, :], 0, 64),
                                 (wpr[1][64:128, :], 64, 128),
                                 (wpr[2][0:64, :], 0, 64)]
                        _idx_emit(nc, work, psp, identf, st[g]["idxall1"],
                                  dests, "w1")
                        posTr = st[g]["posTr"]
                        psb = psp.tile([128, N], F32, tag="ps", name=f"psb_{g}")
                        for c in range(4):
                            nc.tensor.matmul(psb[:, 512 * c:512 * (c + 1)], w1aB2[:],
                                             posTr[:, 512 * c:512 * (c + 1)])
                        B1Td = work.tile([128, N], F32R, tag="B1Td", name=f"B1Td_{g}")
                        nc.scalar.activation(B1Td[:], psb[:], AF.Copy)
                        st[g]["B1Td"] = B1Td

                    chunks.append(s45)

                    for p in range(3):
                        def pa(p=p):
                            sig_pos = _sigma_read(st[g]["posTr"][:])
                            gP = workB.tile([128, N], F32, tag="gslab", bufs=2,
                                            name=f"gP{p}_{g}")
                            nc.gpsimd.ap_gather(
                                out_ap=gP[:].unsqueeze(-1),
                                in_ap=st[g]["B1Td"][:].bitcast(F32).unsqueeze(-1),
                                idxs_ap=wpr[p][:],
                                channels=128, num_elems=N, d=1, num_idxs=N)
                            gPr = workB.tile([128, N], F32R, tag="gslabR",
                                             name=f"gPr{p}_{g}")
                            nc.scalar.activation(gPr[:], gP[:], AF.Copy)
                            psP = psp.tile([128, N], F32, tag="ps", name=f"psP{p}_{g}")
                            for c in range(4):
                                nc.tensor.matmul(psP[:, 512 * c:512 * (c + 1)], w1aA2[:],
                                                 sig_pos[:, 32 * c:32 * (c + 1), :],
                                                 start=True, stop=False)
                            for c in range(4):
                                nc.tensor.matmul(psP[:, 512 * c:512 * (c + 1)], identr[:],
                                                 gPr[:, 512 * c:512 * (c + 1)],
                                                 start=False, stop=True)
                            r1a = work.tile([128, N], F32R, tag="r1aslab",
                                            name=f"r1a{p}_{g}")
                            nc.scalar.activation(r1a[:], psP[:], AF.Relu, bias=b1a2[:])
                            st[g]["r1a"] = r1a

                        def pb(p=p):
                            ps1b = psp.tile([128, N], F32, tag="ps", name=f"ps1b{p}_{g}")
                            for c in range(4):
                                nc.tensor.matmul(ps1b[:, 512 * c:512 * (c + 1)], w1bB[:],
                                                 st[g]["r1a"][:, 512 * c:512 * (c + 1)])
                            r1b = work.tile([128, N], F32R, tag="r1bslab",
                                            name=f"r1b{p}_{g}")
                            nc.scalar.activation(r1b[:], ps1b[:], AF.Relu, bias=b1b2[:])
                            st[g]["r1b"] = r1b

                        def pc(p=p):
                            ps1c = psp.tile([128, N], F32, tag="ps", name=f"ps1c{p}_{g}")
                            for c in range(4):
                                nc.tensor.matmul(ps1c[:, 512 * c:512 * (c + 1)], w1cB[:],
                                                 st[g]["r1b"][:, 512 * c:512 * (c + 1)])
                            if p == 0:
                                mc = workB.tile([128, N], F32, tag="mc0",
                                                name=f"mc{p}_{g}")
                                nc.scalar.activation(mc[:], ps1c[:], AF.Relu,
                                                     bias=b1c2[:])
                                st[g]["macc1"] = mc
                            else:
                                mc = workB.tile([128, N], F32, tag="gslab", bufs=2,
                                                name=f"mc{p}_{g}")
                                nc.scalar.activation(mc[:], ps1c[:], AF.Relu,
                                                     bias=b1c2[:])
                                macc1 = st[g]["macc1"]
                                nc.vector.tensor_tensor(out=macc1[:], in0=macc1[:],
                                                        in1=mc[:], op=ALU.max)

                        chunks.extend([pa, pb, pc])

                    def agg():
                        m2 = st[g]["macc1"]
                        x1tmp = work.tile([64, N], F32, tag="r1aslab",
                                          name=f"x1tmp_{g}")
                        nc.sync.dma_start(x1tmp[:], m2[64:128, :])
                        x1pre = work.tile([64, N], F32, tag="posT0", name=f"x1pre_{g}")
                        nc.vector.tensor_tensor(out=x1pre[:], in0=m2[0:64, :],
                                                in1=x1tmp[:], op=ALU.max)
                        x1nat = work.tile([64, N], F32R, tag="x1nat", bufs=2,
                                          name=f"x1nat_{g}")
                        nc.vector.tensor_scalar(
                            out=_sigma_read(x1nat[:]),
                            in0=x1pre[:].rearrange("c (q b) -> c q b", q=128, b=16),
                            scalar1=s1c[:], scalar2=h1c[:],
                            op0=ALU.mult, op1=ALU.add)
                        st[g]["x1nat"] = x1nat

                    def s7():
                        x1nat = st[g]["x1nat"]
                        L, R = lhs66[g % 2], rhs66[g % 2]
                        sq1 = work.tile([64, N], F32R, tag="posT0", name=f"sq1_{g}")
                        nc.scalar.activation(sq1[:], x1nat[:].bitcast(F32), AF.Square)
                        x2p2n = psp.tile([1, N], F32, tag="ps", name=f"x2p2n_{g}")
                        for c in range(4):
                            nc.tensor.matmul(x2p2n[:, 512 * c:512 * (c + 1)],
                                             pm64[:, 0:1],
                                             sq1[:, 512 * c:512 * (c + 1)])
                        x2p2p = psp.tile([1, N], F32, tag="ps", name=f"x2p2p_{g}")
                        for c in range(4):
                            nc.tensor.matmul(x2p2p[:, 512 * c:512 * (c + 1)],
                                             pm64[:, 1:2],
                                             sq1[:, 512 * c:512 * (c + 1)])
                        nc.scalar.activation(L[0:1, :], x2p2n[:], AF.Copy, scale=0.5)
                        x2r2 = work.tile([64, N], F32R, tag="posT0", name=f"x2r2_{g}")
                        nc.scalar.activation(x2r2[0:1, :], x2p2p[:], AF.Copy, scale=0.5)
                        nc.sync.dma_start(R[65:66, :], x2r2[0:1, :])
                        nc.sync.dma_start(L[1:33, :], x1nat[0:32, :])
                        nc.sync.dma_start(L[33:65, :], x1nat[32:64, :])
                        nc.sync.dma_start(R[1:33, :], x1nat[0:32, :])
                        nc.sync.dma_start(R[33:65, :], x1nat[32:64, :])
                        st[g]["lhs66"], st[g]["rhs66"] = L, R

                    chunks.extend([agg, s7])
                    return chunks

                def b2td_chunk(g):
                    def s9b():
                        x1nat = st[g]["x1nat"]
                        psb2 = psp.tile([128, N], F32, tag="ps", name=f"psb2_{g}")
                        for c in range(4):
                            nc.tensor.matmul(psb2[:, 512 * c:512 * (c + 1)], w2B[:],
                                             x1nat[:, 512 * c:512 * (c + 1)])
                        B2Td = work.tile([128, N], F32R, tag="r1bslab",
                                         name=f"B2Td_{g}")
                        nc.scalar.activation(B2Td[:], psb2[:], AF.Copy)
                        st[g]["B2Td"] = B2Td
                    return s9b

                def conv2_chunks(g, tail=False, skip_b2td=False):
                    chunks = []

                    def s9():
                        dests = [(wrap2[:, kk, :], 0, 128) for kk in range(1, 5)]
                        _idx_emit(nc, work, psp, identf, st[g]["idxall2"],
                                  dests, "w2")

                    chunks.append(s9)
                    if not skip_b2td:
                        chunks.append(b2td_chunk(g))

                    for k in range(K):
                        def gk(k=k):
                            g2 = workB.tile([128, N], F32, tag="gslab", bufs=2,
                                            name=f"g2s{k}_{g}")
                            nc.gpsimd.ap_gather(
                                out_ap=g2[:].unsqueeze(-1),
                                in_ap=st[g]["B2Td"][:].bitcast(F32).unsqueeze(-1),
                                idxs_ap=wrap2[:, k, :],
                                channels=128, num_elems=N, d=1, num_idxs=N)
                            if k == 0:
                                st[g]["g2first"] = g2
                            elif k == 1:
                                macc2 = work.tile([128, N], F32, tag="r1aslab",
                                                  name=f"macc2_{g}")
                                nc.vector.tensor_tensor(
                                    out=macc2[:], in0=st[g]["g2first"][:],
                                    in1=g2[:], op=ALU.max)
                                st[g]["macc2"] = macc2
                            else:
                                macc2 = st[g]["macc2"]
                                nc.vector.tensor_tensor(
                                    out=macc2[:], in0=macc2[:],
                                    in1=g2[:], op=ALU.max)

                        chunks.append(gk)

                    def s12():
                        sig_x1 = _sigma_read(st[g]["x1nat"][:])
                        macc2r = workB.tile([128, N], F32R, tag="gslabR",
                                            name=f"macc2r_{g}")
                        nc.scalar.activation(macc2r[:], st[g]["macc2"][:], AF.Copy)
                        macc2 = macc2r
                        psa2 = psp.tile([128, N], F32, tag="ps", name=f"psa2_{g}")
                        for c in range(4):
                            nc.tensor.matmul(psa2[:, 512 * c:512 * (c + 1)], w2A[:],
                                             sig_x1[:, 32 * c:32 * (c + 1), :],
                                             start=True, stop=False)
                        for c in range(4):
                            nc.tensor.matmul(psa2[:, 512 * c:512 * (c + 1)], identr[:],
                                             macc2[:, 512 * c:512 * (c + 1)],
                                             start=False, stop=True)
                        x2sg = work.tile([128, N], F32R, tag="B1Td", name=f"x2sg_{g}")
                        nc.scalar.activation(x2sg[:], psa2[:], AF.Relu, bias=b2[:])
                        st[g]["x2sg"] = x2sg

                    chunks.append(s12)

                    for mt in range(8):
                        def smt(mt=mt):
                            sig_x1 = _sigma_read(st[g]["x1nat"][:])
                            x2sg = st[g]["x2sg"]
                            psl = psp.tile([128, N], F32, tag="ps",
                                           name=f"psl{mt}_{g}")
                            for c in range(4):
                                nc.tensor.matmul(psl[:, 512 * c:512 * (c + 1)],
                                                 wl1[:, 128 * mt:128 * (mt + 1)],
                                                 sig_x1[:, 32 * c:32 * (c + 1), :],
                                                 start=True, stop=False)
                            for c in range(4):
                                nc.tensor.matmul(psl[:, 512 * c:512 * (c + 1)],
                                                 wl2[:, 128 * mt:128 * (mt + 1)],
                                                 x2sg[:, 512 * c:512 * (c + 1)],
                                                 start=False, stop=True)
                            pr = work.tile([128, 1], F32, tag="poolred", bufs=2,
                                           name=f"pr{mt}_{g}")
                            nc.vector.tensor_reduce(pr[:], psl[:], axis=AX.X,
                                                    op=ALU.max)
                            nc.scalar.activation(poolr[:, mt, g:g + 1], pr[:],
                                                 AF.Relu, bias=bl[:, mt:mt + 1])

                        chunks.append(smt)
                    return chunks

                q = deque()
                p0 = prep_chunks(0)
                p0[0]()
                consts0()
                p0[1]()
                q.extend(wconv)
                q.extend(prep_chunks(1))
                topk_emit(0, q, "lhs5", "rhs5", "idxall1")
                while q:
                    q.popleft()()
                q.extend(conv1_chunks(0))
                topk_emit(1, q, "lhs5", "rhs5", "idxall1")
                while q:
                    q.popleft()()
                q.extend(prep_chunks(2))
                q.extend(conv1_chunks(1))
                topk_emit(0, q, "lhs66", "rhs66", "idxall2")
                while q:
                    q.popleft()()
                q.extend(conv2_chunks(0))
                topk_emit(2, q, "lhs5", "rhs5", "idxall1")
                while q:
                    q.popleft()()
                q.extend(prep_chunks(3))
                q.extend(conv1_chunks(2))
                topk_emit(1, q, "lhs66", "rhs66", "idxall2")
                while q:
                    q.popleft()()
                q.extend(conv2_chunks(1))
                topk_emit(3, q, "lhs5", "rhs5", "idxall1")
                while q:
                    q.popleft()()
                q.extend(conv1_chunks(3))
                topk_emit(2, q, "lhs66", "rhs66", "idxall2")
                while q:
                    q.popleft()()
                q.extend(conv2_chunks(2))
                q.append(b2td_chunk(3))
                topk_emit(3, q, "lhs66", "rhs66", "idxall2")
                while q:
                    q.popleft()()
                for c in conv2_chunks(3, tail=True, skip_b2td=True):
                    c()

            # ============ head MLP (all graphs), scoped pool ============
            with tc.tile_pool(name="headp", bufs=1) as hp:
                wm1 = hp.tile([128, 8, 512], F32, name="wm1s")
                nc.sync.dma_start(wm1[:], wm1_d[:])
                wm2 = hp.tile([128, 4, 256], F32, name="wm2s")
                nc.sync.dma_start(wm2[:], wm2_d[:])
                wout = hp.tile([128, 2, 2], F32, name="wouts")
                nc.sync.dma_start(wout[:], wout_d[:])
                bm1 = hp.tile([128, 4], F32, name="bm1s")
                nc.sync.dma_start(bm1[:], bm1_d[:])
                bm2 = hp.tile([128, 2], F32, name="bm2s")
                nc.sync.dma_start(bm2[:], bm2_d[:])
                bout = hp.tile([2, 1], F32, name="bouts")
                nc.sync.dma_start(bout[:], bout_d[:])

                poolf = hp.tile([128, 8, NG], F32)
                nc.vector.tensor_copy(poolf[:], poolr[:].bitcast(F32))
                rm1 = hp.tile([128, 4, NG], F32)
                for mt in range(4):
                    ph = psp.tile([128, NG], F32, tag="ps")
                    for kk in range(8):
                        nc.tensor.matmul(ph[:], wm1[:, kk, 128 * mt:128 * (mt + 1)],
                                         poolf[:, kk, :], start=(kk == 0), stop=(kk == 7))
                    nc.scalar.activation(rm1[:, mt, :], ph[:], AF.Relu,
                                         bias=bm1[:, mt:mt + 1])
                rm2 = hp.tile([128, 2, NG], F32)
                for mt in range(2):
                    ph = psp.tile([128, NG], F32, tag="ps")
                    for kk in range(4):
                        nc.tensor.matmul(ph[:], wm2[:, kk, 128 * mt:128 * (mt + 1)],
                                         rm1[:, kk, :], start=(kk == 0), stop=(kk == 3))
                    nc.scalar.activation(rm2[:, mt, :], ph[:], AF.Relu,
                                         bias=bm2[:, mt:mt + 1])
                pho = psp.tile([2, NG], F32, tag="ps")
                for kk in range(2):
                    nc.tensor.matmul(pho[:], wout[:, kk, :], rm2[:, kk, :],
                                     start=(kk == 0), stop=(kk == 1))
                outs = hp.tile([2, NG], F32)
                nc.vector.tensor_scalar_add(outs[:], pho[:], bout[:])
                nc.sync.dma_start(out_d[:], outs[:])

    nc.compile()
    return nc


def _idx_emit(nc, work, psp, identf, idxall, dests, tag):
    """Write slab index blocks (k=1..4) straight into gather-table APs.

    For slab k, dest is (ap, lo, hi): partition 16g+b, col q hold neighbor-k
    index of node 128b+q, replicated over g — the replication happens in the
    PE transpose via a 0-step free dim on the read. The copy out of PSUM stays
    on the same partitions (lo:hi) as the transpose output.
    """
    for k in range(4):
        idxf = work.tile([128, 8, 16], F32, tag="idxf",
                         name=f"{tag}idxf{k}")
        nc.vector.tensor_copy(idxf[:, 0, :], idxall[:, :, 1 + k])
        nc.vector.tensor_copy(idxf[:, 1, :], idxf[:, 0, :])
        nc.vector.tensor_copy(idxf[:, 2:4, :], idxf[:, 0:2, :])
        nc.vector.tensor_copy(idxf[:, 4:8, :], idxf[:, 0:4, :])
        tp = psp.tile([128, 128], F32, tag="ps", name=f"{tag}tp{k}")
        nc.tensor.transpose(tp[:], idxf[:].rearrange("p a b -> p (a b)"), identf[:])
        ap, lo, hi = dests[k]
        nc.vector.tensor_copy(ap, tp[lo:hi, :])


def _fold_weights(inp):
    """Host-side BN folding / edge-weight splitting. Layout-only + tiny weight algebra."""
    f = {k: np.asarray(v, dtype=np.float64) for k, v in inp.items()}
    w = {}
    # conv1 layer a: e @ W1a = x_i @ (Wtop - Wbot) + x_j @ Wbot
    w1aA = f["w1a"][:3] - f["w1a"][3:]
    w1aB = f["w1a"][3:]
    w["w1aA2"] = np.concatenate([w1aA, w1aA], axis=1)
    w["w1aB2"] = np.concatenate([w1aB, w1aB], axis=1)
    w["b1a2"] = np.concatenate([f["b1a"], f["b1a"]])
    # fold (s1a, h1a) into layer b; (s1b, h1b) into layer c
    w1b = f["s1a"][:, None] * f["w1b"]
    b1b = f["h1a"] @ f["w1b"] + f["b1b"]
    w1c = f["s1b"][:, None] * f["w1c"]
    b1c = f["h1b"] @ f["w1c"] + f["b1c"]
    z = np.zeros_like(w1b)
    w["w1bB"] = np.block([[w1b, z], [z, w1b]])
    w["w1cB"] = np.block([[w1c, z], [z, w1c]])
    w["b1b2"] = np.concatenate([b1b, b1b])
    w["b1c2"] = np.concatenate([b1c, b1c])
    w["s1c"], w["h1c"] = f["s1c"], f["h1c"]
    # conv2
    w["w2A"] = f["w2"][:64] - f["w2"][64:]
    w["w2B"] = f["w2"][64:]
    w["b2"] = f["b2"]
    # linear l: x1-part plain; x2-part folded with (s2, h2)
    w["wl1"] = f["wl"][:64]
    w["wl2"] = f["s2"][:, None] * f["wl"][64:]
    w["bl"] = f["bl"] + f["h2"] @ f["wl"][64:]
    # head: fold (sl, hl) into m1; (sm1, hm1) into m2; (sm2, hm2) into out
    w["wm1"] = f["sl"][:, None] * f["wm1"]
    w["bm1"] = f["hl"] @ f["wm1"] + f["bm1"]
    w["wm2"] = f["sm1"][:, None] * f["wm2"]
    w["bm2"] = f["hm1"] @ f["wm2"] + f["bm2"]
    w["wout"] = f["sm2"][:, None] * f["wout"]
    w["bout"] = f["hm2"] @ f["wout"] + f["bout"]
    return {k: v.astype(np.float32) for k, v in w.items()}


def _weight_maps(w):
    m = {}
    for k in ["w1aA2", "w1aB2", "w1bB", "w1cB", "w2A", "w2B", "wl1", "wl2"]:
        m[k] = w[k]
    m["wm1"] = np.ascontiguousarray(w["wm1"].reshape(8, 128, 512).transpose(1, 0, 2))
    m["wm2"] = np.ascontiguousarray(w["wm2"].reshape(4, 128, 256).transpose(1, 0, 2))
    m["wout"] = np.ascontiguousarray(w["wout"].reshape(2, 128, 2).transpose(1, 0, 2))
    for k, p in [("b1a2", 128), ("b1b2", 128), ("b1c2", 128), ("s1c", 64),
                 ("h1c", 64), ("b2", 128)]:
        m[k] = w[k].reshape(p, 1)
    m["bl"] = np.ascontiguousarray(w["bl"].reshape(8, 128).T)
    m["bm1"] = np.ascontiguousarray(w["bm1"].reshape(4, 128).T)
    m["bm2"] = np.ascontiguousarray(w["bm2"].reshape(2, 128).T)
    m["bout"] = w["bout"].reshape(2, 1)
    m["ident"] = np.eye(128, dtype=np.float32)
    m["pm3"] = np.stack([-np.ones(3), np.ones(3)], axis=1)
    m["pm64"] = np.stack([-np.ones(64), np.ones(64)], axis=1)
    # iota block: [b, q] -> 128*b + q (sigma self-gather indices)
    b_idx = np.arange(16)[:, None]
    q_idx = np.arange(128)[None, :]
    m["iotaw"] = (128 * b_idx + q_idx).astype(np.int16)
    m["cst"] = np.stack([np.ones(N, np.float32), -np.ones(N, np.float32)])
    return {k: np.ascontiguousarray(v, dtype=(np.int16 if k == "iotaw" else np.float32))
            for k, v in m.items()}


def kernel(**inputs):
    if "nc" not in _CACHE:
        _CACHE["nc"] = build_nc()
    nc = _CACHE["nc"]

    w = _fold_weights(inputs)
    wm = _weight_maps(w)
    pos = np.asarray(inputs["pos"], dtype=np.float32)  # [32, 2048, 3]
    B = pos.shape[0]

    in_maps = []
    for c in range(NCORES):
        m = dict(wm)
        m["posT"] = np.ascontiguousarray(pos[NG * c:NG * (c + 1)].transpose(0, 2, 1))
        in_maps.append(m)

    res = bass_utils.run_bass_kernel_spmd(nc, in_maps, core_ids=list(range(NCORES)))
    out = np.zeros((B, 2), dtype=np.float32)
    for c in range(NCORES):
        out[NG * c:NG * (c + 1)] = res.results[c]["out"].T
    return out
